# revision 8
# baseline (speedup 1.0000x reference)
"""Multi-head attention kernel for Trainium2 (8 NeuronCores).

Problem: inputs query/key/value [2, 64, 64, 256] fp32, NHEAD=8, D=32.
reference: q,k,v -> [N=2, L=4096, H=8, D=32]; softmax(q.k^T/sqrt(D)) @ v.

Sharding: 16 (batch, head) pairs over 8 cores -> each core handles one
batch n = core//4 and two adjacent heads (2*hp, 2*hp+1), hp = core%4, so
its input slice is [4096, 64] contiguous channels.

Per-core algorithm (flash-style, S^T layout, no max subtraction --
logits are ~N(0,1) so exp() is well within fp32 range):
  Q^T, K^T [d=32, 4096] f32r built via PE transposes of [128, 64] slabs,
  emitted just-in-time so MM1s start ~4us in (PE queue is in-order).
  V' [s, 66] = [V_h0 | 1 | V_h1 | 1] bf16 (ones -> softmax denominator).
  For each l-tile (512 cols), each s-tile t (one group = both heads):
    MM1: S^T pair [s=128, 2x512] = K^T_t.T @ Q^T_lt            (PE, f32r)
    exp: split across TWO engines per a fixed per-s-tile pattern:
      ACT: Exp activation (scale=1/sqrt(32)) PSUM -> SBUF bf16
      DVE: Schraudolph exp-as-int-bits: i16 = rint(A*qk + B) PSUM -> SBUF,
           bitcast as bf16 (max ~3% weight err; softmax-averaged impact
           on the output is ~100x smaller -- measured rel err ~6e-3)
    MM2: O'^T [33, 512] += V'_t.T @ expS^T (PE, accum, lagged LAG groups)
  Epilogue per l-tile (all DVE, zero PE):
    one 32x32-block transpose of the [128,512] accumulator (num rows AND
    denominator rows transpose in the same op), reciprocal on the
    transposed denominator columns (FD=16 strided -- 25x cheaper than on
    the [1,512] row), stride-0-broadcast multiply, DMA out.
"""

import numpy as np

L = 4096
D = 32
P = 128
NT = L // P            # 32 s-tiles per head
LT = 512               # l-tile width
N_LT = L // LT         # 8 l-tiles
TEMP = 1.0 / np.sqrt(np.float32(D))
LOG2E = float(np.log2(np.e))

# Schraudolph constants for bf16-bits exp: i16 = rint(A*qk + B)
C_BIAS = -7.0
A_SCH = float(np.float32(TEMP * LOG2E * 128.0))
B_SCH = float(np.float32(127.0 * 128.0 + C_BIAS))

N_DVEU = 26            # units per l-tile computed on DVE (of 64)
N_DVEU0 = 18           # reduced DVE share for l-tile 0 (prologue copies)
LAG = 9                # groups of MM2 held back (software pipeline depth)

_CACHE = {}


_MAXW = 1  # walrus codegen in this container allows 1 sem wait per instruction


def _split_waits_json(bir_json: bytes) -> bytes:
    """Rewrite BIR so no instruction carries more than _MAXW sem waits:
    excess waits move to EventSemaphore carrier instructions inserted
    immediately before, on the same engine (identical blocking semantics)."""
    import json

    m = json.loads(bir_json)
    ctr = 0
    for fn in m.get("functions", []):
        for blk in fn.get("blocks", []):
            out = []
            changed = False
            for ins in blk.get("instructions", []):
                si = ins.get("sync_info")
                waits = si.get("on_wait") if si else None
                if waits and len(waits) > _MAXW:
                    changed = True
                    excess = waits[: -_MAXW]
                    si["on_wait"] = waits[-_MAXW:]
                    for i in range(0, len(excess), _MAXW):
                        ctr += 1
                        out.append(
                            {
                                "debug": ins.get("debug", 0),
                                "engine": ins["engine"],
                                "ins": [],
                                "outs": [],
                                "name": f"EVW-{ctr}",
                                "opcode": "EventSemaphore",
                                "sync_info": {
                                    "on_wait": excess[i : i + _MAXW],
                                    "on_update": [],
                                },
                            }
                        )
                out.append(ins)
            if changed:
                blk["instructions"] = out
    return json.dumps(m).encode()


def _apply_drain_patch():
    """Hook compile_bir_kernel (both the native and the bass2jax/PJRT entry
    points) to run the wait-splitting BIR rewrite before walrus."""
    import concourse.bass_utils as bu

    if getattr(bu, "_ant_split_waits", False):
        return
    orig = bu.compile_bir_kernel

    def wrapped(bir_json, tmpdir, neff_name="file.neff"):
        return orig(_split_waits_json(bir_json), tmpdir, neff_name)

    bu.compile_bir_kernel = wrapped
    bu._ant_split_waits = True
    try:
        import concourse.bass2jax as b2j

        b2j.compile_bir_kernel = wrapped
    except ImportError:
        pass


def _dve_units(n):
    """Bresenham-spread n DVE-assigned units out of 2*NT."""
    NU = 2 * NT
    return {u for u in range(NU) if (u + 1) * n // NU > u * n // NU}


def _build(mode="mixed"):
    import concourse.bass as bass
    import concourse.mybir as mybir
    import concourse.tile as tile
    from concourse.masks import make_identity

    _apply_drain_patch()

    f32 = mybir.dt.float32
    bf16 = mybir.dt.bfloat16
    i16 = mybir.dt.int16
    sdt = mybir.dt.float32r   # QK^T matmul dtype (1 cyc/row, f32-accurate)
    avdt = bf16               # exp-weight / V' dtype for the AV side
    MULT = mybir.AluOpType.mult
    ADD = mybir.AluOpType.add

    nc = bass.Bass("TRN2", debug=False)
    q_d = nc.dram_tensor("q", [L, 64], f32, kind="ExternalInput")
    k_d = nc.dram_tensor("k", [L, 64], f32, kind="ExternalInput")
    v_d = nc.dram_tensor("v", [L, 64], f32, kind="ExternalInput")
    o_d = nc.dram_tensor("o", [L, 64], f32, kind="ExternalOutput")

    with tile.TileContext(nc) as tc:
        with (
            tc.tile_pool(name="const", bufs=1) as const_pool,
            tc.tile_pool(name="slab", bufs=1) as slab_pool,
            tc.tile_pool(name="persist", bufs=1) as persist_pool,
            tc.tile_pool(name="spsum", bufs=5, space="PSUM") as spsum,
            tc.tile_pool(name="apsum", bufs=2, space="PSUM") as apsum,
            tc.tile_pool(name="warm", bufs=1, space="PSUM") as warm_pool,
            tc.tile_pool(name="exps", bufs=6 + 2 * LAG) as exps_pool,
            tc.tile_pool(name="epil", bufs=2) as epil_pool,
        ):
            ident = const_pool.tile([P, P], f32)
            make_identity(nc, ident)
            ones_f = const_pool.tile([P, NT], f32)
            nc.vector.memset(ones_f, 1.0)
            # ACT warmup: trigger the exp table load (~2.7us) during DMA
            warm = const_pool.tile([1, 8], avdt)
            nc.scalar.activation(
                warm[:, :], ident[0:1, 0:8],
                mybir.ActivationFunctionType.Exp, scale=1.0,
            )

            # ---- input DMA, chunked; q/k chunk 0 first for early start --
            qs = slab_pool.tile([P, NT, 64], f32)
            ks = slab_pool.tile([P, NT, 64], f32)
            vs = slab_pool.tile([P, NT, 64], f32)
            q_ap = q_d.ap().rearrange("(t p) c -> p t c", p=P)
            k_ap = k_d.ap().rearrange("(t p) c -> p t c", p=P)
            v_ap = v_d.ap().rearrange("(t p) c -> p t c", p=P)
            order = [(qs, q_ap, 0), (ks, k_ap, 0), (ks, k_ap, 1),
                     (qs, q_ap, 1), (vs, v_ap, 0), (vs, v_ap, 1)]
            for dst_t, src_ap, c2 in order:
                ts_ = slice(c2 * 16, c2 * 16 + 16)
                nc.sync.dma_start(out=dst_t[:, ts_, :], in_=src_ap[:, ts_, :])

            # ---- PE warmup: the HAM clock gate keeps the PE at 1.2 GHz
            # (K=4/8) until it sees ~3.4us of sustained matmul activity;
            # without this the whole kernel runs at half the PE clock.
            # ~40 back-to-back small matmuls on scratch data during the
            # DMA phase push it to 2.4 GHz before the real work starts.
            wsrc = const_pool.tile([P, 128], avdt)
            nc.vector.memset(wsrc, 0.25)
            wps = warm_pool.tile([P, LT], f32)
            for _ in range(40):
                nc.tensor.matmul(
                    wps[:, 0:128], wsrc, wsrc, start=True, stop=True
                )

            # ---- Q^T / K^T transpose groups (emitted just-in-time) ------
            qt = persist_pool.tile([P, L], sdt)
            kt = persist_pool.tile([P, L], sdt)

            def emit_tpose(dst, src, g):
                # strips 0/1 (partitions 0-63) = head0/head1 d-rows;
                # partitions 64-127 replicate them (SBUF->SBUF DMA) so the
                # 4-way row-packed MM1s have disjoint PE row strips.
                tp = spsum.tile([64, 4 * P], f32, tag="sp")
                for j in range(4):
                    t = 4 * g + j
                    nc.tensor.transpose(
                        tp[:, j * P : (j + 1) * P], src[:, t, :], ident
                    )
                csl = slice(g * 512, (g + 1) * 512)
                nc.vector.tensor_copy(out=dst[0:64, csl], in_=tp)
                nc.sync.dma_start(out=dst[64:128, csl], in_=dst[0:64, csl])

            # ---- V' = [v_h0 | 1 | v_h1 | 1] bf16, emitted per chunk -----
            vp = persist_pool.tile([P, NT, 66], avdt)

            def emit_vprep(c2):
                ts_ = slice(c2 * 16, c2 * 16 + 16)
                nc.vector.tensor_copy(out=vp[:, ts_, 0:32], in_=vs[:, ts_, 0:32])
                nc.vector.tensor_copy(out=vp[:, ts_, 33:65], in_=vs[:, ts_, 32:64])
                nc.vector.tensor_copy(
                    out=vp[:, ts_, 32:33], in_=ones_f[:, ts_]
                )
                nc.vector.tensor_copy(
                    out=vp[:, ts_, 65:66], in_=ones_f[:, ts_]
                )

            # ---- main loop ---------------------------------------------
            accum_by_lt = {}
            pend = []
            ex_loc = {}

            def emit_epilogue(lt):
                lsl = slice(lt * LT, (lt + 1) * LT)
                accum = accum_by_lt.pop(lt)
                # 32x32-block transposes with explicit partition placement
                # (v.transpose may cross partitions; other DVE ops cannot):
                # tn rows 0-31 = num_h0^T, 32-63 = num_h1^T; td same for
                # the denominator rows (den in cols 0::32 after transpose)
                tn = epil_pool.tile([64, LT], f32, tag="tn")
                td = epil_pool.tile([64, LT], f32, tag="td")
                nc.vector.transpose(out=tn[0:32, :], in_=accum[0:32, :])
                nc.vector.transpose(out=tn[32:64, :], in_=accum[64:96, :])
                nc.vector.transpose(out=td[0:32, :], in_=accum[32:64, :])
                nc.vector.transpose(out=td[32:64, :], in_=accum[96:128, :])
                rcd = epil_pool.tile([64, 16], f32, tag="rcd")
                nc.vector.reciprocal(out=rcd[0:32, :], in_=td[0:32, 0::32])
                nc.vector.reciprocal(out=rcd[32:64, :], in_=td[32:64, 0::32])
                oN = epil_pool.tile([64, LT], f32, tag="oN")
                nc.vector.tensor_tensor(
                    out=oN[:, :].rearrange("p (b d) -> p b d", d=32),
                    in0=tn[:, :].rearrange("p (b d) -> p b d", d=32),
                    in1=rcd[:, :].broadcast_to((64, 16, 32)),
                    op=MULT,
                )
                for h in (0, 1):
                    nc.sync.dma_start(
                        out=o_d.ap()[lsl, 32 * h : 32 * h + 32].rearrange(
                            "(blk p) d -> p blk d", p=32
                        ),
                        in_=oN[32 * h : 32 * h + 32, :].rearrange(
                            "p (blk d) -> p blk d", d=32
                        ),
                    )

            def flush_mm2(limit):
                while pend and len(pend) > limit:
                    lt, t, ex0, ex1 = pend.pop(0)
                    if lt not in accum_by_lt:
                        accum = apsum.tile([P, LT], f32, tag="accum")
                        accum_by_lt[lt] = accum
                    accum = accum_by_lt[lt]
                    st_f = dict(start=(t == 0), stop=(t == NT - 1))
                    # rows 0-32: h0 num+den; rows 64-96: h1 (col-packed)
                    nc.tensor.matmul(
                        accum[0:33, :], vp[:, t, 0:33], ex0,
                        tile_position=(0, 0), **st_f,
                    )
                    nc.tensor.matmul(
                        accum[64:97, :], vp[:, t, 33:66], ex1,
                        tile_position=(0, 64), **st_f,
                    )
                    if t == NT - 1:
                        emit_epilogue(lt)

            def emit_unit(lt, t, h, on_dve, nfill):
                lsl = slice(lt * LT, (lt + 1) * LT)
                # Keep-warm fillers: dependency-free matmuls into the
                # dedicated warm PSUM bank.  They bridge PE-idle stretches
                # (exp-pipeline fill, engine jitter) so the HAM clock gate
                # keeps the PE at 2.4 GHz instead of falling to 1.2 GHz.
                for _ in range(nfill):
                    nc.tensor.matmul(
                        wps[:, 0:128], wsrc, wsrc, start=True, stop=True
                    )
                sp = spsum.tile([P, LT], f32, tag="sp")
                ex = exps_pool.tile([P, LT], avdt, tag="ex")
                st = 32 * ((2 * t + h) % 4)
                nc.tensor.matmul(
                    sp[:, :],
                    kt[st : st + 32, t * P : (t + 1) * P],
                    qt[st : st + 32, lsl],
                    start=True,
                    stop=True,
                    tile_position=(st, 0),
                )
                if on_dve:
                    nc.vector.tensor_scalar(
                        out=ex[:, :].bitcast(i16),
                        in0=sp[:, :],
                        scalar1=A_SCH,
                        scalar2=B_SCH,
                        op0=MULT,
                        op1=ADD,
                    )
                else:
                    nc.scalar.activation(
                        ex[:, :], sp[:, :],
                        mybir.ActivationFunctionType.Exp,
                        scale=float(TEMP),
                    )
                ex_loc[(lt, t, h)] = ex[:, :]
                if (lt, t, 0) in ex_loc and (lt, t, 1) in ex_loc:
                    pend.append(
                        (lt, t, ex_loc.pop((lt, t, 0)), ex_loc.pop((lt, t, 1)))
                    )
                flush_mm2(LAG)

            emit_tpose(qt, qs, 0)
            emit_tpose(kt, ks, 0)
            dve0, dve = _dve_units(N_DVEU0), _dve_units(N_DVEU)
            for lt in range(N_LT):
                for t in range(NT):
                    for h in (0, 1):
                        if lt == 0 and h == 0:
                            if 1 <= t <= 7:
                                emit_tpose(kt, ks, t)   # K groups 1..7 JIT
                            if t in (9, 12):
                                emit_vprep((t - 9) // 3)  # V' chunks 0..1
                        if t == 16 and h == 0 and lt < N_LT - 1:
                            emit_tpose(qt, qs, lt + 1)  # next l-tile's Q^T
                        u = 2 * t + h
                        nfill = 8 if (lt == 0 and u < 24) else 1
                        emit_unit(
                            lt, t, h,
                            u in (dve0 if lt == 0 else dve), nfill,
                        )
                    if lt == N_LT - 1 and t > NT - LAG:
                        flush_mm2(max(0, NT - 1 - t))
            flush_mm2(0)
    return nc


def _get_nc(mode):
    if mode not in _CACHE:
        _CACHE[mode] = _build(mode)
    return _CACHE[mode]


def kernel(query, key, value, mode="mixed", trace=False):
    from concourse.bass_utils import run_bass_kernel_spmd

    q = np.ascontiguousarray(np.asarray(query, np.float32)).reshape(2, L, 256)
    k = np.ascontiguousarray(np.asarray(key, np.float32)).reshape(2, L, 256)
    v = np.ascontiguousarray(np.asarray(value, np.float32)).reshape(2, L, 256)

    nc = _get_nc(mode)
    in_maps = []
    for c in range(8):
        n, hp = divmod(c, 4)
        sl = slice(64 * hp, 64 * hp + 64)
        in_maps.append(
            {
                "q": np.ascontiguousarray(q[n, :, sl]),
                "k": np.ascontiguousarray(k[n, :, sl]),
                "v": np.ascontiguousarray(v[n, :, sl]),
            }
        )
    kwargs = {}
    if trace:
        kwargs = dict(trace=True)
    res = run_bass_kernel_spmd(nc, in_maps, core_ids=list(range(8)), **kwargs)
    out = np.zeros((2, L, 8, 32), np.float32)
    for c, r in enumerate(res.results):
        n, hp = divmod(c, 4)
        out[n, :, 2 * hp : 2 * hp + 2, :] = r["o"].reshape(L, 2, 32)
    if trace:
        return out, res
    return out


# revision 9
# speedup vs baseline: 2.0496x; 2.0496x over previous
"""Multi-head attention kernel for Trainium2 (8 NeuronCores).

Problem: inputs query/key/value [2, 64, 64, 256] fp32, NHEAD=8, D=32.
reference: q,k,v -> [N=2, L=4096, H=8, D=32]; softmax(q.k^T/sqrt(D)) @ v.

Sharding: 16 (batch, head) pairs over 8 cores -> each core handles one
batch n = core//4 and two adjacent heads (2*hp, 2*hp+1), hp = core%4, so
its input slice is [4096, 64] contiguous channels.

Per-core algorithm (flash-style, S^T layout, no max subtraction --
logits are ~N(0,1) so exp() is well within fp32 range):
  Q^T, K^T [d=32, 4096] f32r built via PE transposes of [128, 64] slabs,
  emitted just-in-time so MM1s start ~4us in (PE queue is in-order).
  V' [s, 66] = [V_h0 | 1 | V_h1 | 1] bf16 (ones -> softmax denominator).
  For each l-tile (512 cols), each s-tile t (one group = both heads):
    MM1: S^T pair [s=128, 2x512] = K^T_t.T @ Q^T_lt            (PE, f32r)
    exp: split across TWO engines per a fixed per-s-tile pattern:
      ACT: Exp activation (scale=1/sqrt(32)) PSUM -> SBUF bf16
      DVE: Schraudolph exp-as-int-bits: i16 = rint(A*qk + B) PSUM -> SBUF,
           bitcast as bf16 (max ~3% weight err; softmax-averaged impact
           on the output is ~100x smaller -- measured rel err ~6e-3)
    MM2: O'^T [33, 512] += V'_t.T @ expS^T (PE, accum, lagged LAG groups)
  Epilogue per l-tile (all DVE, zero PE):
    one 32x32-block transpose of the [128,512] accumulator (num rows AND
    denominator rows transpose in the same op), reciprocal on the
    transposed denominator columns (FD=16 strided -- 25x cheaper than on
    the [1,512] row), stride-0-broadcast multiply, DMA out.
"""

import numpy as np

L = 4096
D = 32
P = 128
NT = L // P            # 32 s-tiles per head
LT = 512               # l-tile width
N_LT = L // LT         # 8 l-tiles
TEMP = 1.0 / np.sqrt(np.float32(D))
LOG2E = float(np.log2(np.e))

# Schraudolph constants for bf16-bits exp: i16 = rint(A*qk + B)
C_BIAS = -7.0
A_SCH = float(np.float32(TEMP * LOG2E * 128.0))
B_SCH = float(np.float32(127.0 * 128.0 + C_BIAS))

N_DVE = 13             # s-tiles per l-tile computed on DVE (of 32)
N_DVE0 = 9             # reduced DVE share for l-tile 0 (prologue copies)
LAG = 9                # groups of MM2 held back (software pipeline depth)

_CACHE = {}


_MAXW = 1  # walrus codegen in this container allows 1 sem wait per instruction


def _split_waits_json(bir_json: bytes) -> bytes:
    """Rewrite BIR so no instruction carries more than _MAXW sem waits:
    excess waits move to EventSemaphore carrier instructions inserted
    immediately before, on the same engine (identical blocking semantics)."""
    import json

    m = json.loads(bir_json)
    ctr = 0
    for fn in m.get("functions", []):
        for blk in fn.get("blocks", []):
            out = []
            changed = False
            for ins in blk.get("instructions", []):
                si = ins.get("sync_info")
                waits = si.get("on_wait") if si else None
                if waits and len(waits) > _MAXW:
                    changed = True
                    excess = waits[: -_MAXW]
                    si["on_wait"] = waits[-_MAXW:]
                    for i in range(0, len(excess), _MAXW):
                        ctr += 1
                        out.append(
                            {
                                "debug": ins.get("debug", 0),
                                "engine": ins["engine"],
                                "ins": [],
                                "outs": [],
                                "name": f"EVW-{ctr}",
                                "opcode": "EventSemaphore",
                                "sync_info": {
                                    "on_wait": excess[i : i + _MAXW],
                                    "on_update": [],
                                },
                            }
                        )
                out.append(ins)
            if changed:
                blk["instructions"] = out
    return json.dumps(m).encode()


def _apply_drain_patch():
    """Hook compile_bir_kernel (both the native and the bass2jax/PJRT entry
    points) to run the wait-splitting BIR rewrite before walrus."""
    import concourse.bass_utils as bu

    if getattr(bu, "_ant_split_waits", False):
        return
    orig = bu.compile_bir_kernel

    def wrapped(bir_json, tmpdir, neff_name="file.neff"):
        return orig(_split_waits_json(bir_json), tmpdir, neff_name)

    bu.compile_bir_kernel = wrapped
    bu._ant_split_waits = True
    try:
        import concourse.bass2jax as b2j

        b2j.compile_bir_kernel = wrapped
    except ImportError:
        pass


def _dve_tiles(n):
    """Bresenham-spread n DVE-assigned s-tiles out of NT."""
    return {t for t in range(NT) if (t + 1) * n // NT > t * n // NT}


def _build(mode="mixed"):
    import concourse.bass as bass
    import concourse.mybir as mybir
    import concourse.tile as tile
    from concourse.masks import make_identity

    _apply_drain_patch()

    f32 = mybir.dt.float32
    bf16 = mybir.dt.bfloat16
    i16 = mybir.dt.int16
    sdt = mybir.dt.float32r   # QK^T matmul dtype (1 cyc/row, f32-accurate)
    avdt = bf16               # exp-weight / V' dtype for the AV side
    MULT = mybir.AluOpType.mult
    ADD = mybir.AluOpType.add

    nc = bass.Bass("TRN2", debug=False)
    q_d = nc.dram_tensor("q", [L, 64], f32, kind="ExternalInput")
    k_d = nc.dram_tensor("k", [L, 64], f32, kind="ExternalInput")
    v_d = nc.dram_tensor("v", [L, 64], f32, kind="ExternalInput")
    o_d = nc.dram_tensor("o", [L, 64], f32, kind="ExternalOutput")

    with tile.TileContext(nc) as tc:
        with (
            tc.tile_pool(name="const", bufs=1) as const_pool,
            tc.tile_pool(name="slab", bufs=1) as slab_pool,
            tc.tile_pool(name="persist", bufs=1) as persist_pool,
            tc.tile_pool(name="spsum", bufs=3, space="PSUM") as spsum,
            tc.tile_pool(name="apsum", bufs=1, space="PSUM") as apsum,
            tc.tile_pool(name="warm", bufs=1, space="PSUM") as warm_pool,
            tc.tile_pool(name="exps", bufs=3 + LAG) as exps_pool,
            tc.tile_pool(name="epil", bufs=2) as epil_pool,
        ):
            ident = const_pool.tile([P, P], f32)
            make_identity(nc, ident)
            ones_f = const_pool.tile([P, NT], f32)
            nc.vector.memset(ones_f, 1.0)
            # ACT warmup: trigger the exp table load (~2.7us) during DMA
            warm = const_pool.tile([1, 8], avdt)
            nc.scalar.activation(
                warm[:, :], ident[0:1, 0:8],
                mybir.ActivationFunctionType.Exp, scale=1.0,
            )

            # ---- input DMA, chunked; q/k chunk 0 first for early start --
            qs = slab_pool.tile([P, NT, 64], f32)
            ks = slab_pool.tile([P, NT, 64], f32)
            vs = slab_pool.tile([P, NT, 64], f32)
            q_ap = q_d.ap().rearrange("(t p) c -> p t c", p=P)
            k_ap = k_d.ap().rearrange("(t p) c -> p t c", p=P)
            v_ap = v_d.ap().rearrange("(t p) c -> p t c", p=P)
            order = [(qs, q_ap, 0), (ks, k_ap, 0), (ks, k_ap, 1),
                     (qs, q_ap, 1), (vs, v_ap, 0), (vs, v_ap, 1)]
            for dst_t, src_ap, c2 in order:
                ts_ = slice(c2 * 16, c2 * 16 + 16)
                nc.sync.dma_start(out=dst_t[:, ts_, :], in_=src_ap[:, ts_, :])

            # ---- PE warmup: the HAM clock gate keeps the PE at 1.2 GHz
            # (K=4/8) until it sees ~3.4us of sustained matmul activity;
            # without this the whole kernel runs at half the PE clock.
            # ~40 back-to-back small matmuls on scratch data during the
            # DMA phase push it to 2.4 GHz before the real work starts.
            wsrc = const_pool.tile([P, 128], avdt)
            nc.vector.memset(wsrc, 0.25)
            wps = warm_pool.tile([P, LT], f32)
            for _ in range(40):
                nc.tensor.matmul(
                    wps[:, 0:128], wsrc, wsrc, start=True, stop=True
                )

            # ---- Q^T / K^T transpose groups (emitted just-in-time) ------
            qt = persist_pool.tile([P, L], sdt)
            kt = persist_pool.tile([P, L], sdt)

            def emit_tpose(dst, src, g):
                # strips 0/1 (partitions 0-63) = head0/head1 d-rows;
                # partitions 64-127 replicate them (SBUF->SBUF DMA) so the
                # 4-way row-packed MM1s have disjoint PE row strips.
                tp = spsum.tile([64, 4 * P], f32, tag="sp")
                for j in range(4):
                    t = 4 * g + j
                    nc.tensor.transpose(
                        tp[:, j * P : (j + 1) * P], src[:, t, :], ident
                    )
                csl = slice(g * 512, (g + 1) * 512)
                nc.vector.tensor_copy(out=dst[0:64, csl], in_=tp)
                nc.sync.dma_start(out=dst[64:128, csl], in_=dst[0:64, csl])

            # ---- V' = [v_h0 | 1 | v_h1 | 1] bf16, emitted per chunk -----
            vp = persist_pool.tile([P, NT, 66], avdt)

            def emit_vprep(c2):
                ts_ = slice(c2 * 16, c2 * 16 + 16)
                nc.vector.tensor_copy(out=vp[:, ts_, 0:32], in_=vs[:, ts_, 0:32])
                nc.vector.tensor_copy(out=vp[:, ts_, 33:65], in_=vs[:, ts_, 32:64])
                nc.vector.tensor_copy(
                    out=vp[:, ts_, 32:33], in_=ones_f[:, ts_]
                )
                nc.vector.tensor_copy(
                    out=vp[:, ts_, 65:66], in_=ones_f[:, ts_]
                )

            # ---- main loop ---------------------------------------------
            accum_by_lt = {}
            pend = []

            def emit_epilogue(lt):
                lsl = slice(lt * LT, (lt + 1) * LT)
                accum = accum_by_lt.pop(lt)
                # 32x32-block transposes with explicit partition placement
                # (v.transpose may cross partitions; other DVE ops cannot):
                # tn rows 0-31 = num_h0^T, 32-63 = num_h1^T; td same for
                # the denominator rows (den in cols 0::32 after transpose)
                tn = epil_pool.tile([64, LT], f32, tag="tn")
                td = epil_pool.tile([64, LT], f32, tag="td")
                nc.vector.transpose(out=tn[0:32, :], in_=accum[0:32, :])
                nc.vector.transpose(out=tn[32:64, :], in_=accum[64:96, :])
                nc.vector.transpose(out=td[0:32, :], in_=accum[32:64, :])
                nc.vector.transpose(out=td[32:64, :], in_=accum[96:128, :])
                rcd = epil_pool.tile([64, 16], f32, tag="rcd")
                nc.vector.reciprocal(out=rcd[0:32, :], in_=td[0:32, 0::32])
                nc.vector.reciprocal(out=rcd[32:64, :], in_=td[32:64, 0::32])
                oN = epil_pool.tile([64, LT], f32, tag="oN")
                nc.vector.tensor_tensor(
                    out=oN[:, :].rearrange("p (b d) -> p b d", d=32),
                    in0=tn[:, :].rearrange("p (b d) -> p b d", d=32),
                    in1=rcd[:, :].broadcast_to((64, 16, 32)),
                    op=MULT,
                )
                for h in (0, 1):
                    nc.sync.dma_start(
                        out=o_d.ap()[lsl, 32 * h : 32 * h + 32].rearrange(
                            "(blk p) d -> p blk d", p=32
                        ),
                        in_=oN[32 * h : 32 * h + 32, :].rearrange(
                            "p (blk d) -> p blk d", d=32
                        ),
                    )

            def flush_mm2(limit):
                while pend and len(pend) > limit:
                    lt, t, ex0, ex1 = pend.pop(0)
                    if lt not in accum_by_lt:
                        accum = apsum.tile([P, LT], f32, tag="accum")
                        accum_by_lt[lt] = accum
                    accum = accum_by_lt[lt]
                    st_f = dict(start=(t == 0), stop=(t == NT - 1))
                    # rows 0-32: h0 num+den; rows 64-96: h1 (col-packed)
                    nc.tensor.matmul(
                        accum[0:33, :], vp[:, t, 0:33], ex0,
                        tile_position=(0, 0), **st_f,
                    )
                    nc.tensor.matmul(
                        accum[64:97, :], vp[:, t, 33:66], ex1,
                        tile_position=(0, 64), **st_f,
                    )
                    if t == NT - 1:
                        emit_epilogue(lt)

            def emit_group(lt, t, on_dve, nfill):
                lsl = slice(lt * LT, (lt + 1) * LT)
                # Keep-warm fillers: dependency-free matmuls into the
                # dedicated warm PSUM bank.  They bridge PE-idle stretches
                # (exp-pipeline fill, engine jitter) so the HAM clock gate
                # keeps the PE at 2.4 GHz instead of falling to 1.2 GHz.
                for _ in range(nfill):
                    nc.tensor.matmul(
                        wps[:, 0:128], wsrc, wsrc, start=True, stop=True
                    )
                sp = spsum.tile([P, 2 * LT], f32, tag="sp")
                ex = exps_pool.tile([P, 2 * LT], avdt, tag="ex")
                for h in (0, 1):
                    st = 32 * ((2 * t + h) % 4)
                    nc.tensor.matmul(
                        sp[:, h * LT : (h + 1) * LT],
                        kt[st : st + 32, t * P : (t + 1) * P],
                        qt[st : st + 32, lsl],
                        start=True,
                        stop=True,
                        tile_position=(st, 0),
                    )
                if on_dve:
                    nc.vector.tensor_scalar(
                        out=ex[:, :].bitcast(i16),
                        in0=sp[:, :],
                        scalar1=A_SCH,
                        scalar2=B_SCH,
                        op0=MULT,
                        op1=ADD,
                    )
                else:
                    nc.scalar.activation(
                        ex[:, :], sp[:, :],
                        mybir.ActivationFunctionType.Exp,
                        scale=float(TEMP),
                    )
                pend.append((lt, t, ex[:, 0:LT], ex[:, LT : 2 * LT]))
                flush_mm2(LAG)

            emit_tpose(qt, qs, 0)
            emit_tpose(kt, ks, 0)
            dve0, dve = _dve_tiles(N_DVE0), _dve_tiles(N_DVE)
            for lt in range(N_LT):
                for t in range(NT):
                    if lt == 0:
                        if 1 <= t <= 7:
                            emit_tpose(kt, ks, t)       # K groups 1..7 JIT
                        if t in (9, 12):
                            emit_vprep((t - 9) // 3)    # V' chunks 0..1
                    if t == 16 and lt < N_LT - 1:
                        emit_tpose(qt, qs, lt + 1)      # next l-tile's Q^T
                    nfill = 16 if (lt == 0 and t < 12) else 1
                    emit_group(lt, t, t in (dve0 if lt == 0 else dve), nfill)
                    if lt == N_LT - 1 and t > NT - LAG:
                        flush_mm2(max(0, NT - 1 - t))
            flush_mm2(0)
    return nc


def _get_nc(mode):
    if mode not in _CACHE:
        _CACHE[mode] = _build(mode)
    return _CACHE[mode]


def kernel(query, key, value, mode="mixed", trace=False):
    from concourse.bass_utils import run_bass_kernel_spmd

    q = np.ascontiguousarray(np.asarray(query, np.float32)).reshape(2, L, 256)
    k = np.ascontiguousarray(np.asarray(key, np.float32)).reshape(2, L, 256)
    v = np.ascontiguousarray(np.asarray(value, np.float32)).reshape(2, L, 256)

    nc = _get_nc(mode)
    in_maps = []
    for c in range(8):
        n, hp = divmod(c, 4)
        sl = slice(64 * hp, 64 * hp + 64)
        in_maps.append(
            {
                "q": np.ascontiguousarray(q[n, :, sl]),
                "k": np.ascontiguousarray(k[n, :, sl]),
                "v": np.ascontiguousarray(v[n, :, sl]),
            }
        )
    kwargs = {}
    if trace:
        kwargs = dict(trace=True)
    res = run_bass_kernel_spmd(nc, in_maps, core_ids=list(range(8)), **kwargs)
    out = np.zeros((2, L, 8, 32), np.float32)
    for c, r in enumerate(res.results):
        n, hp = divmod(c, 4)
        out[n, :, 2 * hp : 2 * hp + 2, :] = r["o"].reshape(L, 2, 32)
    if trace:
        return out, res
    return out
